# revision 1
# baseline (speedup 1.0000x reference)
"""Trainium2 Bass kernel for nn_L2MLoRA (fused linear + routed LoRA).

Math (per batch element b, with e = idx[b,0]):
    y[b] = x[b] @ W.T + bias + SCALE * (x[b] @ A_pool[e]) @ B_pool[e]

Strategy: data-parallel over batch B=8 -> one batch element per NeuronCore.
The expert gather (A_pool[e], B_pool[e]) happens on host, so each core gets
exactly one [DIM, RANK] / [RANK, DIM] expert pair. Everything is computed in
the transposed domain (yT = W @ xT + ...) so all matmul operands already have
the contraction dim on partitions and no on-device transposes are needed:

    yT[o, t]  = sum_d W[o,d] * xT[d,t] + bias[o] + sum_r B2[r,o] * rT[r,t]
    rT[r, t]  = sum_d A[d,r] * xT[d,t]          (B2 = SCALE * B_pool[e])

PE matmuls run in float32r (fp32 bits, 1 cycle/row at N>=256 vs 4 for fp32).
Bias is applied by ScalarE during the PSUM->SBUF copy.
"""

import numpy as np

import concourse.bass as bass
import concourse.tile as tile
from concourse import bacc, mybir
from concourse.bass_utils import run_bass_kernel_spmd

B, N, DIM, POOL, RANK = 8, 2048, 1024, 64, 8
SCALE = 2.0
NCORES = 8
P = 128          # partitions / k-tile height / o-chunk width
TW = 512         # token-chunk width (max f32 moving free dim = PSUM bank)
KT = DIM // P    # 8 k-tiles over the contraction dim
OT = DIM // P    # 8 output chunks
TT = N // TW     # 4 token chunks
F32 = mybir.dt.float32
F32R = mybir.dt.float32r


def build_program(n_iter: int = 1, probe: str = "full"):
    """Build the single-core Tile program (same program runs SPMD on 8 cores).

    n_iter > 1 wraps the body in a For_i loop for benchmarking.
    probe: "full" | "nodma" (x resident, no stores) | "dmaonly" (no matmuls).
    """
    nc = bacc.Bacc("TRN2", target_bir_lowering=False, debug=False,
                   num_devices=NCORES)

    x_d = nc.dram_tensor("xt", [KT, TT, P, TW], F32R, kind="ExternalInput")
    w_d = nc.dram_tensor("wt", [OT, P, KT * P], F32R, kind="ExternalInput")
    a_d = nc.dram_tensor("ap", [P, KT * RANK], F32R, kind="ExternalInput")
    b_d = nc.dram_tensor("bp", [RANK, DIM], F32R, kind="ExternalInput")
    bias_d = nc.dram_tensor("bias", [P, OT], F32, kind="ExternalInput")
    y_d = nc.dram_tensor("y", [TT, P, OT, TW], F32, kind="ExternalOutput")

    with tile.TileContext(nc) as tc:
        with (
            tc.tile_pool(name="cpool", bufs=1) as cpool,
            tc.tile_pool(name="xpool", bufs=(32 if probe == "nodma" else 16)) as xpool,
            tc.tile_pool(name="rpool", bufs=2) as rpool,
            tc.tile_pool(name="opool", bufs=2) as opool,
            tc.tile_pool(name="psy", bufs=6, space="PSUM") as psy_pool,
            tc.tile_pool(name="psr", bufs=2, space="PSUM") as psr_pool,
        ):
            def load_xt(t):
                tiles = []
                for k in range(KT):
                    xx = xpool.tile([P, TW], F32R, tag="xx")
                    nc.sync.dma_start(xx[:], x_d.ap()[k, t])
                    tiles.append(xx)
                return tiles

            # Constants: loaded once, persist across benchmark iterations.
            # Small tensors first, then (for the single-shot program) the
            # t=0 x tiles ahead of the 4MB weight load so PE starts early.
            a_sb = cpool.tile([P, KT * RANK], F32R, tag="a")
            nc.sync.dma_start(a_sb[:], a_d.ap()[:])
            bias_sb = cpool.tile([P, OT], F32, tag="bias")
            nc.sync.dma_start(bias_sb[:], bias_d.ap()[:])
            b_sb = cpool.tile([RANK, DIM], F32R, tag="b")
            nc.sync.dma_start(b_sb[:], b_d.ap()[:])
            first_tiles = load_xt(0) if (n_iter == 1 and probe != "nodma") else None
            w_sb = []
            for o in range(OT):
                w = cpool.tile([P, KT * P], F32R, tag=f"w{o}")
                nc.sync.dma_start(w[:], w_d.ap()[o])
                w_sb.append(w)

            if probe == "nodma":
                resident = [load_xt(t) for t in range(TT)]

            def body(xt_cur=None):
                if probe != "nodma" and xt_cur is None:
                    xt_cur = load_xt(0)
                for t in range(TT):
                    if probe == "nodma":
                        xt = resident[t]
                        xt_next = None
                    else:
                        # Prefetch next t-chunk BEFORE this chunk's compute /
                        # store sits on the in-order SP queue.
                        xt_next = load_xt(t + 1) if t + 1 < TT else None
                        xt = xt_cur

                    ob = opool.tile([P, OT, TW], F32, tag="ob")
                    if probe != "dmaonly":
                        # rT[r, t] = A.T @ xT  (accumulate over k-tiles)
                        ps_r = psr_pool.tile([RANK, TW], F32)
                        for k in range(KT):
                            nc.tensor.matmul(
                                ps_r[:],
                                a_sb[:, k * RANK:(k + 1) * RANK],
                                xt[k][:],
                                start=(k == 0), stop=(k == KT - 1),
                            )
                        r_sb = rpool.tile([RANK, TW], F32R)
                        nc.vector.tensor_copy(r_sb[:], ps_r[:])

                        for o in range(OT):
                            ps = psy_pool.tile([P, TW], F32)
                            for k in range(KT):
                                nc.tensor.matmul(
                                    ps[:],
                                    w_sb[o][:, k * P:(k + 1) * P],
                                    xt[k][:],
                                    start=(k == 0), stop=False,
                                )
                            # low-rank correction into same PSUM accumulation
                            nc.tensor.matmul(
                                ps[:],
                                b_sb[:, o * P:(o + 1) * P],
                                r_sb[:],
                                start=False, stop=True,
                            )
                            nc.scalar.activation(
                                ob[:, o, :], ps[:],
                                mybir.ActivationFunctionType.Identity,
                                bias=bias_sb[:, o:o + 1], scale=1.0,
                            )
                    if probe != "nodma":
                        # one contiguous 2MB store per t-chunk
                        nc.sync.dma_start(y_d.ap()[t], ob[:])
                    xt_cur = xt_next

            if n_iter == 1:
                body(first_tiles)
            else:
                with tc.For_i(0, n_iter, 1,
                              hint_engines=tuple(mybir.ALL_ENGINES)):
                    body()

    nc.compile()
    return nc


def _round_fp32r(a):
    """Round fp32 to the PE's FP32R storage format: 1-8-11, RNE, low 12
    mantissa bits zero (walrus fp32_to_fp32r keeps the top 20 bits)."""
    u = np.ascontiguousarray(a, dtype=np.float32).view(np.uint32)
    r = (u + np.uint32(0x7FF) + ((u >> np.uint32(12)) & np.uint32(1))) & np.uint32(
        0xFFFFF000
    )
    return r.view(np.float32)


def make_in_maps(x, idx, weight, bias, A_pool, B_pool):
    """Host-side shard + relayout. Returns per-core input dicts."""
    x = np.asarray(x, dtype=np.float32)
    idx = np.asarray(idx)
    weight = np.asarray(weight, dtype=np.float32)
    bias = np.asarray(bias, dtype=np.float32)
    A_pool = np.asarray(A_pool, dtype=np.float32)
    B_pool = np.asarray(B_pool, dtype=np.float32)

    # W[o, d] -> wt[o_chunk, p(=d within k), k*128 + c(=o within chunk)]
    wt = _round_fp32r(
        weight.reshape(OT, P, KT, P).transpose(0, 3, 2, 1).reshape(OT, P, KT * P)
    )
    bias_t = np.ascontiguousarray(bias.reshape(OT, P).T)  # [p, o_chunk]

    sel = idx.reshape(B).astype(np.int64)
    in_maps = []
    for c in range(NCORES):
        xT = x[c].T  # [DIM, N]
        xt = _round_fp32r(xT.reshape(KT, P, TT, TW).transpose(0, 2, 1, 3))
        A = A_pool[sel[c]]  # [DIM, RANK]
        ap = _round_fp32r(
            A.reshape(KT, P, RANK).transpose(1, 0, 2).reshape(P, KT * RANK)
        )
        bp = _round_fp32r(SCALE * B_pool[sel[c]])  # [RANK, DIM]
        in_maps.append({"xt": xt, "wt": wt, "ap": ap, "bp": bp, "bias": bias_t})
    return in_maps


def assemble_output(results):
    """Per-core y blocks [OT, TT, P, TW] -> full [B, N, DIM] output."""
    out = np.empty((B, N, DIM), dtype=np.float32)
    for c in range(NCORES):
        yb = results[c]["y"]  # [TT, P, OT, TW]; yb[t,p,o,j] = y[c, t*TW+j, o*P+p]
        out[c] = yb.transpose(0, 3, 2, 1).reshape(N, DIM)
    return out


_PROGRAM_CACHE = {}


def _get_program(n_iter: int = 1):
    if n_iter not in _PROGRAM_CACHE:
        _PROGRAM_CACHE[n_iter] = build_program(n_iter)
    return _PROGRAM_CACHE[n_iter]


def kernel(x, idx, frozen_mask, weight, bias, A_pool, B_pool):
    # frozen_mask only affects gradients (stop_gradient); forward is identical.
    nc = _get_program(1)
    in_maps = make_in_maps(x, idx, weight, bias, A_pool, B_pool)
    res = run_bass_kernel_spmd(nc, in_maps, list(range(NCORES)))
    return assemble_output(res.results)



# revision 2
# speedup vs baseline: 2.3527x; 2.3527x over previous
"""Trainium2 Bass kernel for nn_L2MLoRA (fused linear + routed LoRA).

Math (per batch element b, with e = idx[b,0]):
    y[b] = x[b] @ W.T + bias + SCALE * (x[b] @ A_pool[e]) @ B_pool[e]

Strategy: data-parallel over batch B=8 -> one batch element per NeuronCore.
The expert gather (A_pool[e], B_pool[e]) happens on host, so each core gets
exactly one [DIM, RANK] / [RANK, DIM] expert pair. Because there is a single
expert per core, the LoRA term is folded into the base weight ON DEVICE once
at setup:

    W'stat[d, o] = Wstat[d, o] + SCALE * sum_r A[d, r] * B[r, o]

(16 rank-8 matmuls + 64 vector adds, outside the steady-state loop), after
which every iteration is a pure GEMM + bias:

    yT[o, t] = sum_d W'[o, d] * xT[d, t] + bias[o]

Everything is computed in the transposed domain (yT = W' @ xT) so matmul
operands already have the contraction dim on partitions and no on-device
transposes are needed. Inputs/outputs move in bf16 (PE rate is identical to
fp32r, HBM traffic halves: 4MB x in + 2MB W + 4MB y out per core); PSUM
accumulation stays fp32 and the host converts y back to fp32.
"""

import numpy as np

import concourse.bass as bass
import concourse.tile as tile
from concourse import bacc, mybir
from concourse.bass_utils import run_bass_kernel_spmd

B, N, DIM, POOL, RANK = 8, 2048, 1024, 64, 8
SCALE = 2.0
NCORES = 8
P = 128          # partitions / k-tile height / o-chunk width
TW = 512         # token-chunk width (PSUM bank = 512 fp32 free elems)
KT = DIM // P    # 8 k-tiles over the contraction dim
OT = DIM // P    # 8 output chunks
TT = N // TW     # 4 token chunks
KP = KT * P      # 1024
F32 = mybir.dt.float32
BF16 = mybir.dt.bfloat16


def build_program(n_iter: int = 1, probe: str = "full"):
    """Build the single-core Tile program (same program runs SPMD on 8 cores).

    n_iter > 1 wraps the body in a For_i loop for benchmarking.
    probe: "full" | "nodma" (x resident, no stores) | "dmaonly" (no matmuls).
    """
    nc = bacc.Bacc("TRN2", target_bir_lowering=False, debug=False,
                   num_devices=NCORES)

    # x:  xt[t, p, k*TW + j] = x[t*TW + j, k*128 + p]   (one 1MB DMA per chunk)
    x_d = nc.dram_tensor("xt", [TT, P, KT * TW], BF16, kind="ExternalInput")
    # W:  wt[p, o*KP + k*128 + c] = W[o*128 + c, k*128 + p]  (stationary layout)
    w_d = nc.dram_tensor("wt", [P, OT * KP], BF16, kind="ExternalInput")
    # A^T pre-transposed:  at[r, d] = A[d, r]
    at_d = nc.dram_tensor("at", [RANK, DIM], BF16, kind="ExternalInput")
    # SCALE * B:  bp[r, o*128 + c] = SCALE * B[r, o*128 + c]
    bp_d = nc.dram_tensor("bp", [RANK, DIM], BF16, kind="ExternalInput")
    bias_d = nc.dram_tensor("bias", [P, OT], F32, kind="ExternalInput")
    # y: y[t, c, o*TW + j] = y_full[t*TW + j, o*128 + c]
    y_d = nc.dram_tensor("y", [TT, P, OT * TW], BF16, kind="ExternalOutput")

    with tile.TileContext(nc) as tc:
        with (
            tc.tile_pool(name="cpool", bufs=1) as cpool,
            tc.tile_pool(name="xpool", bufs=(TT + 1 if probe == "nodma" else 4)) as xpool,
            tc.tile_pool(name="opool", bufs=2) as opool,
            tc.tile_pool(name="psy", bufs=6, space="PSUM") as psy_pool,
            tc.tile_pool(name="psd", bufs=2, space="PSUM") as psd_pool,
        ):
            def load_xt(t):
                xx = xpool.tile([P, KT * TW], BF16, tag="xx")
                nc.sync.dma_start(xx[:], x_d.ap()[t])
                return xx

            # Constants: loaded once, persist across benchmark iterations.
            at_sb = cpool.tile([RANK, DIM], BF16, tag="at")
            nc.sync.dma_start(at_sb[:], at_d.ap()[:])
            bp_sb = cpool.tile([RANK, DIM], BF16, tag="bp")
            nc.sync.dma_start(bp_sb[:], bp_d.ap()[:])
            bias_sb = cpool.tile([P, OT], F32, tag="bias")
            nc.sync.dma_start(bias_sb[:], bias_d.ap()[:])
            first_tile = load_xt(0) if (n_iter == 1 and probe != "nodma") else None
            w_sb = cpool.tile([P, OT * KP], BF16, tag="w")
            nc.sync.dma_start(w_sb[:], w_d.ap()[:])

            # One-time fold: W' = W + SCALE * (A @ B) in the stationary layout.
            # delta_stat[d, o] = sum_r at[r, d] * bp[r, o], done per k-chunk of
            # d (128 partitions) x half of o (512 free).
            for k in range(KT):
                for h in range(2):
                    ps = psd_pool.tile([P, TW], F32)
                    nc.tensor.matmul(
                        ps[:],
                        at_sb[:, k * P:(k + 1) * P],
                        bp_sb[:, h * TW:(h + 1) * TW],
                        start=True, stop=True,
                    )
                    for oo in range(4):
                        o = h * 4 + oo
                        w_slice = w_sb[:, o * KP + k * P: o * KP + (k + 1) * P]
                        nc.vector.tensor_add(
                            w_slice, w_slice, ps[:, oo * P:(oo + 1) * P]
                        )

            if probe == "nodma":
                resident = [load_xt(t) for t in range(TT)]

            def body(xt_cur=None):
                if probe != "nodma" and xt_cur is None:
                    xt_cur = load_xt(0)
                for t in range(TT):
                    if probe == "nodma":
                        xt = resident[t]
                        xt_next = None
                    else:
                        # Prefetch next t-chunk BEFORE this chunk's compute /
                        # store sits on the in-order SP queue.
                        xt_next = load_xt(t + 1) if t + 1 < TT else None
                        xt = xt_cur

                    ob = opool.tile([P, OT, TW], BF16, tag="ob")
                    if probe != "dmaonly":
                        for o in range(OT):
                            ps = psy_pool.tile([P, TW], F32)
                            for k in range(KT):
                                nc.tensor.matmul(
                                    ps[:],
                                    w_sb[:, o * KP + k * P: o * KP + (k + 1) * P],
                                    xt[:, k * TW:(k + 1) * TW],
                                    start=(k == 0), stop=(k == KT - 1),
                                )
                            nc.scalar.activation(
                                ob[:, o, :], ps[:],
                                mybir.ActivationFunctionType.Identity,
                                bias=bias_sb[:, o:o + 1], scale=1.0,
                            )
                    if probe != "nodma":
                        # one contiguous 1MB store per t-chunk
                        nc.sync.dma_start(y_d.ap()[t], ob[:])
                    xt_cur = xt_next

            if n_iter == 1:
                body(first_tile)
            else:
                with tc.For_i(0, n_iter, 1,
                              hint_engines=tuple(mybir.ALL_ENGINES)):
                    body()

    nc.compile()
    return nc


def make_in_maps(x, idx, weight, bias, A_pool, B_pool):
    """Host-side shard + relayout. Returns per-core input dicts."""
    bf16 = mybir.dt.np(BF16)
    x = np.asarray(x, dtype=np.float32)
    idx = np.asarray(idx)
    weight = np.asarray(weight, dtype=np.float32)
    bias = np.asarray(bias, dtype=np.float32)
    A_pool = np.asarray(A_pool, dtype=np.float32)
    B_pool = np.asarray(B_pool, dtype=np.float32)

    # W[o, d] -> wt[p(=d within k), o*KP + k*128 + c(=o within chunk)]
    wt = np.ascontiguousarray(
        weight.reshape(OT, P, KT, P).transpose(3, 0, 2, 1).reshape(P, OT * KP)
    ).astype(bf16)
    bias_t = np.ascontiguousarray(bias.reshape(OT, P).T)  # [p, o_chunk]

    sel = idx.reshape(B).astype(np.int64)
    in_maps = []
    for c in range(NCORES):
        # x[n, d] -> xt[t, p, k*TW + j]
        xt = np.ascontiguousarray(
            x[c].reshape(TT, TW, KT, P).transpose(0, 3, 2, 1).reshape(TT, P, KT * TW)
        ).astype(bf16)
        at = np.ascontiguousarray(A_pool[sel[c]].T).astype(bf16)   # [RANK, DIM]
        bp = (SCALE * B_pool[sel[c]]).astype(bf16)                 # [RANK, DIM]
        in_maps.append({"xt": xt, "wt": wt, "at": at, "bp": bp, "bias": bias_t})
    return in_maps


def assemble_output(results):
    """Per-core y blocks [TT, P, OT*TW] -> full [B, N, DIM] fp32 output."""
    out = np.empty((B, N, DIM), dtype=np.float32)
    for c in range(NCORES):
        yb = np.asarray(results[c]["y"]).astype(np.float32)
        # yb[t, c_, o*TW + j] = y[c, t*TW + j, o*128 + c_]
        out[c] = yb.reshape(TT, P, OT, TW).transpose(0, 3, 2, 1).reshape(N, DIM)
    return out


_PROGRAM_CACHE = {}


def _get_program(n_iter: int = 1):
    if n_iter not in _PROGRAM_CACHE:
        _PROGRAM_CACHE[n_iter] = build_program(n_iter)
    return _PROGRAM_CACHE[n_iter]


def kernel(x, idx, frozen_mask, weight, bias, A_pool, B_pool):
    # frozen_mask only affects gradients (stop_gradient); forward is identical.
    nc = _get_program(1)
    in_maps = make_in_maps(x, idx, weight, bias, A_pool, B_pool)
    res = run_bass_kernel_spmd(nc, in_maps, list(range(NCORES)))
    return assemble_output(res.results)
